# revision 13
# baseline (speedup 1.0000x reference)
"""Trainium2 Bass kernel for batched dot-product attention.

Problem: query/keys/values [4, 4096, 64] fp32 -> softmax(Q K^T / 8) V.

Sharding: 8 cores, data parallel. Core c handles batch c//2, query rows
(c%2)*2048 : (c%2+1)*2048, and needs full K/V of its batch. Each core runs
an identical program (SPMD) on its own shard.

Per-core algorithm (flash-attention-like, no max subtraction -- scores are
~N(0,1) after the 1/sqrt(64) scale so exp() cannot overflow):
  - K^T [64, 4096] and Q^T [64, 2048] built via PE transposes (contraction
    dim d must sit on partitions for the QK^T matmul).
  - V' = [V | ones] chunks [128, 65]: the ones column makes the second
    matmul emit the softmax denominator as row 64 of the output.
  - For each 512-wide q tile: 32 matmuls produce scoresT [k=128, q=512]
    chunks in PSUM; ScalarE applies exp (scale folded in); second matmul
    accumulates V'^T @ P into out^T [65, 512]; epilogue transposes back to
    [q, 65], divides by the denominator column and DMAs out.
"""

import math

import numpy as np

import concourse.bass as bass
import concourse.tile as tile
from concourse import bacc, mybir
from concourse.masks import make_identity

FP32 = mybir.dt.float32

# Production shard dims
B, LQ_FULL, LK, D = 4, 4096, 4096, 64
N_CORES = 8
LQ = LQ_FULL * B // N_CORES  # 2048 query rows per core


def emit_attention(tc, q, k, v, o, LQ, LK, D, QTW):
    """Emit the per-core attention program into TileContext tc.

    q: [LQ, D], k: [LK, D], v: [LK, D] DRAM inputs; o: [LQ, D] DRAM output.
    QTW: q-tile width (free dim of the scores matmul), <= 512 for fp32.
    """
    nc = tc.nc
    NKC = LK // 128   # key chunks
    NQC = LQ // 128   # query chunks (for transposes)
    NQT = LQ // QTW   # q tiles
    DP = D + 1        # V width with the ones column
    SCALE = 1.0 / math.sqrt(D)
    Exp = mybir.ActivationFunctionType.Exp

    from contextlib import ExitStack

    with ExitStack() as ctx:
        const = ctx.enter_context(tc.tile_pool(name="const", bufs=1))
        kq = ctx.enter_context(tc.tile_pool(name="kq", bufs=1))
        loadp = ctx.enter_context(tc.tile_pool(name="load", bufs=1))
        pp = ctx.enter_context(tc.tile_pool(name="p", bufs=4))
        outp = ctx.enter_context(tc.tile_pool(name="outs", bufs=4))
        ps_scores = ctx.enter_context(
            tc.tile_pool(name="ps_scores", bufs=2, space="PSUM")
        )
        ps_acc = ctx.enter_context(tc.tile_pool(name="ps_acc", bufs=2, space="PSUM"))
        ps_tr = ctx.enter_context(tc.tile_pool(name="ps_tr", bufs=2, space="PSUM"))

        ident = const.tile([128, 128], FP32)
        make_identity(nc, ident[:])

        # V' = [V | 1] chunks, [128, DP] each, packed side by side.
        # One DMA for all of V, one strided memset for the ones columns.
        vp = const.tile([128, NKC * DP], FP32)
        vp3 = vp[:].rearrange("p (c w) -> p c w", w=DP)
        nc.sync.dma_start(vp3[:, :, 0:D], v.rearrange("(c p) d -> p c d", p=128))
        nc.gpsimd.memset(vp3[:, :, D:DP], 1.0)

        # Natural-layout K/Q staged chunk-major with one DMA each.
        kn = loadp.tile([128, NKC * D], FP32, tag="kn")
        nc.sync.dma_start(
            kn[:].rearrange("p (c d) -> p c d", d=D),
            k.rearrange("(c p) d -> p c d", p=128),
        )
        qn = loadp.tile([128, NQC * D], FP32, tag="qn")
        nc.sync.dma_start(
            qn[:].rearrange("p (c d) -> p c d", d=D),
            q.rearrange("(c p) d -> p c d", p=128),
        )

        # K^T / Q^T via PE transposes of the natural-layout chunks.
        kt = kq.tile([D, LK], FP32)
        for c in range(NKC):
            tr = ps_scores.tile([D, 128], FP32, tag="scores")
            nc.tensor.transpose(tr[:], kn[:, c * D : (c + 1) * D], ident[:])
            nc.vector.tensor_copy(kt[:, c * 128 : (c + 1) * 128], tr[:])

        qt = kq.tile([D, LQ], FP32)
        for c in range(NQC):
            tr = ps_scores.tile([D, 128], FP32, tag="scores")
            nc.tensor.transpose(tr[:], qn[:, c * D : (c + 1) * D], ident[:])
            nc.vector.tensor_copy(qt[:, c * 128 : (c + 1) * 128], tr[:])

        for ti in range(NQT):
            acc = ps_acc.tile([DP, QTW], FP32)
            npair = NKC // 2
            for pi in range(npair):
                c0, c1 = 2 * pi, 2 * pi + 1
                qs = qt[:, ti * QTW : (ti + 1) * QTW]
                s = ps_scores.tile([128, 2 * QTW], FP32, tag="scores")
                nc.tensor.matmul(
                    s[:, :QTW], kt[:, c0 * 128 : (c0 + 1) * 128], qs,
                    start=True, stop=True,
                )
                nc.tensor.matmul(
                    s[:, QTW:], kt[:, c1 * 128 : (c1 + 1) * 128], qs,
                    start=True, stop=True,
                )
                p = pp.tile([128, 2 * QTW], FP32, tag="p")
                nc.scalar.activation(p[:], s[:], Exp, scale=SCALE)
                nc.tensor.matmul(
                    acc[:], vp[:, c0 * DP : (c0 + 1) * DP], p[:, :QTW],
                    start=(pi == 0), stop=False,
                )
                nc.tensor.matmul(
                    acc[:], vp[:, c1 * DP : (c1 + 1) * DP], p[:, QTW:],
                    start=False, stop=(pi == npair - 1),
                )

            # Epilogue: transpose out^T back to [q, DP], divide by denominator.
            accs = outp.tile([DP, QTW], FP32, tag="accs")
            nc.vector.tensor_copy(accs[:], acc[:])
            for j in range(QTW // 128):
                tr = ps_tr.tile([128, DP], FP32)
                nc.tensor.transpose(tr[:], accs[:, j * 128 : (j + 1) * 128],
                                    ident[:DP, :DP])
                rc = outp.tile([128, 1], FP32, tag="rc")
                nc.vector.reciprocal(rc[:], tr[:, D:DP])
                ot = outp.tile([128, D], FP32, tag="ot")
                nc.vector.tensor_scalar_mul(ot[:], tr[:, :D], rc[:])
                r0 = ti * QTW + j * 128
                nc.sync.dma_start(o[r0 : r0 + 128, :], ot[:])


H16 = mybir.dt.float16


def emit_attention_v2(tc, q, k, v, o, LQ, LK, D, QTW):
    """v2: bf16 K^T/Q^T via DMA xbar transposes in a packed layout
    (even k-chunks on partitions 0-63, odd on 64-127) + row-tiled
    concurrent scores matmuls + 3-chunk exp groups.

    Requires D == 64 (pair-packing trick) and QTW == 512.
    """
    nc = tc.nc
    assert D == 64 and QTW % 128 == 0
    NKC = LK // 128   # k chunks
    NQC = LQ // 128   # q chunks
    NQT = LQ // QTW
    DP = D + 1
    GRP = 3           # chunks per exp group (3 PSUM banks)
    SCALE = 1.0 / math.sqrt(D)
    Exp = mybir.ActivationFunctionType.Exp

    from contextlib import ExitStack

    with ExitStack() as ctx:
        const = ctx.enter_context(tc.tile_pool(name="const", bufs=1))
        loadp = ctx.enter_context(tc.tile_pool(name="load", bufs=1))
        pp = ctx.enter_context(tc.tile_pool(name="p", bufs=4))
        outp = ctx.enter_context(tc.tile_pool(name="outs", bufs=4))
        ps_scores = ctx.enter_context(
            tc.tile_pool(name="ps_scores", bufs=2, space="PSUM")
        )
        ps_acc = ctx.enter_context(tc.tile_pool(name="ps_acc", bufs=1, space="PSUM"))
        ps_tr = ctx.enter_context(tc.tile_pool(name="ps_tr", bufs=1, space="PSUM"))

        ident = const.tile([128, 128], FP32)
        make_identity(nc, ident[:])

        # V' = [V | 1] chunks (fp32; value path stays fp32).
        vp = const.tile([128, NKC * DP], FP32)
        vp3 = vp[:].rearrange("p (c w) -> p c w", w=DP)
        nc.sync.dma_start(vp3[:, :, 0:D], v.rearrange("(c p) d -> p c d", p=128))
        nc.gpsimd.memset(vp3[:, :, D:DP], 1.0)

        # K: load natural chunk-major, cast to bf16, xbar-transpose pair
        # blocks. Transposing kb[:, j*128:(j+1)*128] (= chunks 2j | 2j+1 side
        # by side) yields K^T of chunk 2j on partitions 0-63 and of chunk
        # 2j+1 on partitions 64-127 -- the packed row-tiling layout.
        kn = loadp.tile([128, NKC * D], FP32, tag="kn")
        nc.sync.dma_start(
            kn[:].rearrange("p (c d) -> p c d", d=D),
            k.rearrange("(c p) d -> p c d", p=128),
        )
        kb = loadp.tile([128, NKC * D], H16, tag="kb")
        nc.vector.tensor_copy(kb[:], kn[:])
        kt2 = const.tile([128, NKC * D], H16)
        for j in range(NKC // 2):
            nc.sync.dma_start(
                kt2[:, j * 128 : (j + 1) * 128],
                kb[:, j * 128 : (j + 1) * 128],
                transpose=True,
            )

        # Q: same transpose trick into alternating halves, then rearrange
        # into qt2r with Q^T replicated on both partition halves.
        qn = loadp.tile([128, NQC * D], FP32, tag="qn")
        nc.sync.dma_start(
            qn[:].rearrange("p (c d) -> p c d", d=D),
            q.rearrange("(c p) d -> p c d", p=128),
        )
        qb = loadp.tile([128, NQC * D], H16, tag="qb")
        nc.vector.tensor_copy(qb[:], qn[:])
        qt_alt = loadp.tile([128, NQC * D], H16, tag="qt_alt")
        for j in range(NQC // 2):
            nc.sync.dma_start(
                qt_alt[:, j * 128 : (j + 1) * 128],
                qb[:, j * 128 : (j + 1) * 128],
                transpose=True,
            )
        qt2r = const.tile([128, LQ], H16)
        nblk = NQC // 2
        for parity in range(2):
            src = qt_alt[parity * 64 : (parity + 1) * 64, :].rearrange(
                "p (j c) -> p j c", c=128
            )
            for half in range(2):
                dst = qt2r[half * 64 : (half + 1) * 64, :].rearrange(
                    "p (j c) -> p j c", c=256
                )[:, :, parity * 128 : (parity + 1) * 128]
                nc.sync.dma_start(dst, src)

        for ti in range(NQT):
            acc = ps_acc.tile([DP, QTW], FP32)
            ngrp = (NKC + GRP - 1) // GRP
            s_tiles = [None] * ngrp
            filled = [0] * ngrp
            qsl = slice(ti * QTW, (ti + 1) * QTW)

            def consume_group(g):
                gsz = min(GRP, NKC - g * GRP)
                p = pp.tile([128, GRP * QTW], FP32, tag="p")
                nc.scalar.activation(
                    p[:, : gsz * QTW], s_tiles[g][:, : gsz * QTW], Exp, scale=SCALE
                )
                for s in range(gsz):
                    c = g * GRP + s
                    nc.tensor.matmul(
                        acc[:],
                        vp[:, c * DP : (c + 1) * DP],
                        p[:, s * QTW : (s + 1) * QTW],
                        start=(c == 0),
                        stop=(c == NKC - 1),
                    )

            for c in range(NKC):
                g, slot = divmod(c, GRP)
                if s_tiles[g] is None:
                    s_tiles[g] = ps_scores.tile(
                        [128, GRP * QTW], FP32, tag="scores", name="sgrp"
                    )
                half, j = c % 2, c // 2
                nc.tensor.matmul(
                    s_tiles[g][:, slot * QTW : (slot + 1) * QTW],
                    kt2[half * 64 : (half + 1) * 64, j * 128 : (j + 1) * 128],
                    qt2r[half * 64 : (half + 1) * 64, qsl],
                    start=True,
                    stop=True,
                )
                filled[g] += 1
                if filled[g] == min(GRP, NKC - g * GRP):
                    consume_group(g)

            # Epilogue: transpose out^T back to [q, DP], divide by denom.
            accs = outp.tile([DP, QTW], FP32, tag="accs")
            nc.vector.tensor_copy(accs[:], acc[:])
            for j in range(QTW // 128):
                tr = ps_tr.tile([128, DP], FP32)
                nc.tensor.transpose(tr[:], accs[:, j * 128 : (j + 1) * 128],
                                    ident[:DP, :DP])
                rc = outp.tile([128, 1], FP32, tag="rc")
                nc.vector.reciprocal(rc[:], tr[:, D:DP])
                ot = outp.tile([128, D], FP32, tag="ot")
                nc.vector.tensor_scalar_mul(ot[:], tr[:, :D], rc[:])
                r0 = ti * QTW + j * 128
                nc.sync.dma_start(o[r0 : r0 + 128, :], ot[:])


_BUILT = {}

EMITTERS = {"v1": emit_attention, "v2": emit_attention_v2}
DEFAULT_VARIANT = "v1"


def _build(repeat=1, variant=None):
    """Build the per-core module. repeat>1 wraps the body in a hardware
    For_i loop (used only for on-device timing; grading uses repeat=1)."""
    variant = variant or DEFAULT_VARIANT
    key = (repeat, variant)
    if key not in _BUILT:
        emitter = EMITTERS[variant]
        nc = bacc.Bacc("TRN2", target_bir_lowering=False, debug=False)
        q = nc.dram_tensor("q", [LQ, D], FP32, kind="ExternalInput")
        k = nc.dram_tensor("k", [LK, D], FP32, kind="ExternalInput")
        v = nc.dram_tensor("v", [LK, D], FP32, kind="ExternalInput")
        o = nc.dram_tensor("o", [LQ, D], FP32, kind="ExternalOutput")
        with tile.TileContext(nc) as tc:
            if repeat == 1:
                emitter(tc, q[:], k[:], v[:], o[:], LQ, LK, D, QTW=512)
            else:
                engines = [
                    mybir.EngineType.PE,
                    mybir.EngineType.Activation,
                    mybir.EngineType.DVE,
                    mybir.EngineType.SP,
                    mybir.EngineType.Pool,
                ]
                with tc.For_i(0, repeat, 1, hint_engines=tuple(engines)):
                    emitter(tc, q[:], k[:], v[:], o[:], LQ, LK, D, QTW=512)
        nc.compile()
        _BUILT[key] = nc
    return _BUILT[key]


def _shard_inputs(query, keys, values):
    in_maps = []
    for c in range(N_CORES):
        b, h = c // 2, c % 2
        in_maps.append({
            "q": np.ascontiguousarray(query[b, h * LQ : (h + 1) * LQ, :],
                                      dtype=np.float32),
            "k": np.ascontiguousarray(keys[b], dtype=np.float32),
            "v": np.ascontiguousarray(values[b], dtype=np.float32),
        })
    return in_maps


def run_sharded(query, keys, values, trace=False, repeat=1, variant=None):
    """Run on 8 cores; returns (full_output, BassKernelResults)."""
    from concourse.bass_utils import run_bass_kernel_spmd

    nc = _build(repeat, variant)
    in_maps = _shard_inputs(query, keys, values)
    res = run_bass_kernel_spmd(nc, in_maps, list(range(N_CORES)), trace=trace)
    out = np.empty((B, LQ_FULL, D), np.float32)
    for c in range(N_CORES):
        b, h = c // 2, c % 2
        out[b, h * LQ : (h + 1) * LQ, :] = res.results[c]["o"]
    return out, res


def kernel(query, keys, values):
    out, _ = run_sharded(np.asarray(query), np.asarray(keys), np.asarray(values))
    return out


# revision 36
# speedup vs baseline: 1.6564x; 1.6564x over previous
"""Trainium2 Bass kernel for batched dot-product attention.

Problem: query/keys/values [4, 4096, 64] fp32 -> softmax(Q K^T / 8) V.

Sharding: 8 cores, data parallel. Core c handles batch c//2, query rows
(c%2)*2048 : (c%2+1)*2048, and needs full K/V of its batch. Each core runs
an identical program (SPMD) on its own shard.

Per-core algorithm (flash-attention-like, no max subtraction -- scores are
~N(0,1) after the 1/sqrt(64) scale so exp() cannot overflow):
  - K^T [64, 4096] and Q^T [64, 2048] built via PE transposes (contraction
    dim d must sit on partitions for the QK^T matmul).
  - V' = [V | ones] chunks [128, 65]: the ones column makes the second
    matmul emit the softmax denominator as row 64 of the output.
  - For each 512-wide q tile: 32 matmuls produce scoresT [k=128, q=512]
    chunks in PSUM; ScalarE applies exp (scale folded in); second matmul
    accumulates V'^T @ P into out^T [65, 512]; epilogue transposes back to
    [q, 65], divides by the denominator column and DMAs out.
"""

import math

import numpy as np

import concourse.bass as bass
import concourse.tile as tile
from concourse import bacc, mybir
from concourse.masks import make_identity

FP32 = mybir.dt.float32

# Production shard dims
B, LQ_FULL, LK, D = 4, 4096, 4096, 64
N_CORES = 8
LQ = LQ_FULL * B // N_CORES  # 2048 query rows per core


def emit_attention(tc, q, k, v, o, LQ, LK, D, QTW, GRP=2, setup_only=False):
    """Emit the per-core attention program into TileContext tc.

    q: [LQ, D], k: [LK, D], v: [LK, D] DRAM inputs; o: [LQ, D] DRAM output.
    QTW: q-tile width (free dim of the scores matmul), <= 512 for fp32.
    GRP: k-chunks per exp group (2 -> 4+2+2 PSUM banks, 3 -> 6+1+1).
    """
    nc = tc.nc
    NKC = LK // 128   # key chunks
    NQC = LQ // 128   # query chunks (for transposes)
    NQT = LQ // QTW   # q tiles
    DP = D + 1        # V width with the ones column
    SCALE = 1.0 / math.sqrt(D)
    Exp = mybir.ActivationFunctionType.Exp
    accbufs = 2 if GRP == 2 else 1

    from contextlib import ExitStack

    with ExitStack() as ctx:
        const = ctx.enter_context(tc.tile_pool(name="const", bufs=1))
        kq = ctx.enter_context(tc.tile_pool(name="kq", bufs=1))
        loadp = ctx.enter_context(tc.tile_pool(name="load", bufs=1))
        pp = ctx.enter_context(tc.tile_pool(name="p", bufs=4))
        outp = ctx.enter_context(tc.tile_pool(name="outs", bufs=4))
        ps_scores = ctx.enter_context(
            tc.tile_pool(name="ps_scores", bufs=2, space="PSUM")
        )
        ps_acc = ctx.enter_context(
            tc.tile_pool(name="ps_acc", bufs=accbufs, space="PSUM")
        )
        ps_tr = ctx.enter_context(
            tc.tile_pool(name="ps_tr", bufs=accbufs, space="PSUM")
        )

        ident = const.tile([128, 128], FP32)
        make_identity(nc, ident[:])

        # V' = [V | 1] chunks, [128, DP] each, packed side by side.
        # One DMA for all of V, one strided memset for the ones columns.
        vp = const.tile([128, NKC * DP], FP32)
        vp3 = vp[:].rearrange("p (c w) -> p c w", w=DP)
        nc.sync.dma_start(vp3[:, :, 0:D], v.rearrange("(c p) d -> p c d", p=128))
        nc.gpsimd.memset(vp3[:, :, D:DP], 1.0)

        # Natural-layout K/Q staged chunk-major with one DMA each.
        kn = loadp.tile([128, NKC * D], FP32, tag="kn")
        nc.sync.dma_start(
            kn[:].rearrange("p (c d) -> p c d", d=D),
            k.rearrange("(c p) d -> p c d", p=128),
        )
        qn = loadp.tile([128, NQC * D], FP32, tag="qn")
        nc.sync.dma_start(
            qn[:].rearrange("p (c d) -> p c d", d=D),
            q.rearrange("(c p) d -> p c d", p=128),
        )

        # K^T / Q^T via PE transposes of the natural-layout chunks.
        kt = kq.tile([D, LK], FP32)
        for c in range(NKC):
            tr = ps_scores.tile([D, 128], FP32, tag="scores")
            nc.tensor.transpose(tr[:], kn[:, c * D : (c + 1) * D], ident[:])
            nc.vector.tensor_copy(kt[:, c * 128 : (c + 1) * 128], tr[:])

        qt = kq.tile([D, LQ], FP32)
        for c in range(NQC):
            tr = ps_scores.tile([D, 128], FP32, tag="scores")
            nc.tensor.transpose(tr[:], qn[:, c * D : (c + 1) * D], ident[:])
            nc.vector.tensor_copy(qt[:, c * 128 : (c + 1) * 128], tr[:])

        if setup_only:
            return

        for ti in range(NQT):
            acc = ps_acc.tile([DP, QTW], FP32)
            ngrp = (NKC + GRP - 1) // GRP
            qs = qt[:, ti * QTW : (ti + 1) * QTW]
            for g in range(ngrp):
                gsz = min(GRP, NKC - g * GRP)
                s = ps_scores.tile([128, GRP * QTW], FP32, tag="scores")
                for i in range(gsz):
                    c = g * GRP + i
                    nc.tensor.matmul(
                        s[:, i * QTW : (i + 1) * QTW],
                        kt[:, c * 128 : (c + 1) * 128], qs,
                        start=True, stop=True,
                    )
                p = pp.tile([128, GRP * QTW], FP32, tag="p")
                nc.scalar.activation(
                    p[:, : gsz * QTW], s[:, : gsz * QTW], Exp, scale=SCALE
                )
                for i in range(gsz):
                    c = g * GRP + i
                    nc.tensor.matmul(
                        acc[:], vp[:, c * DP : (c + 1) * DP],
                        p[:, i * QTW : (i + 1) * QTW],
                        start=(c == 0), stop=(c == NKC - 1),
                    )

            # Epilogue: transpose out^T back to [q, DP], divide by denominator.
            accs = outp.tile([DP, QTW], FP32, tag="accs")
            nc.vector.tensor_copy(accs[:], acc[:])
            for j in range(QTW // 128):
                tr = ps_tr.tile([128, DP], FP32)
                nc.tensor.transpose(tr[:], accs[:, j * 128 : (j + 1) * 128],
                                    ident[:DP, :DP])
                rc = outp.tile([128, 1], FP32, tag="rc")
                nc.vector.reciprocal(rc[:], tr[:, D:DP])
                ot = outp.tile([128, D], FP32, tag="ot")
                nc.vector.tensor_scalar_mul(ot[:], tr[:, :D], rc[:])
                r0 = ti * QTW + j * 128
                nc.sync.dma_start(o[r0 : r0 + 128, :], ot[:])


H16 = mybir.dt.float16


def emit_attention_v2(tc, q, k, v, o, LQ, LK, D, QTW, GRP=3, setup_only=False):
    """v2: bf16 K^T/Q^T via DMA xbar transposes in a packed layout
    (even k-chunks on partitions 0-63, odd on 64-127) + row-tiled
    concurrent scores matmuls + 3-chunk exp groups.

    Requires D == 64 (pair-packing trick) and QTW == 512.
    """
    nc = tc.nc
    assert D == 64 and QTW % 128 == 0
    NKC = LK // 128   # k chunks
    NQC = LQ // 128   # q chunks
    NQT = LQ // QTW
    DP = D + 1
    GRP = 3           # chunks per exp group (3 PSUM banks)
    SCALE = 1.0 / math.sqrt(D)
    Exp = mybir.ActivationFunctionType.Exp

    from contextlib import ExitStack

    with ExitStack() as ctx:
        const = ctx.enter_context(tc.tile_pool(name="const", bufs=1))
        loadp = ctx.enter_context(tc.tile_pool(name="load", bufs=1))
        pp = ctx.enter_context(tc.tile_pool(name="p", bufs=4))
        outp = ctx.enter_context(tc.tile_pool(name="outs", bufs=4))
        ps_scores = ctx.enter_context(
            tc.tile_pool(name="ps_scores", bufs=2, space="PSUM")
        )
        ps_acc = ctx.enter_context(tc.tile_pool(name="ps_acc", bufs=1, space="PSUM"))
        ps_tr = ctx.enter_context(tc.tile_pool(name="ps_tr", bufs=1, space="PSUM"))

        ident = const.tile([128, 128], FP32)
        make_identity(nc, ident[:])

        # V' = [V | 1] chunks (fp32; value path stays fp32).
        vp = const.tile([128, NKC * DP], FP32)
        vp3 = vp[:].rearrange("p (c w) -> p c w", w=DP)
        nc.sync.dma_start(vp3[:, :, 0:D], v.rearrange("(c p) d -> p c d", p=128))
        nc.gpsimd.memset(vp3[:, :, D:DP], 1.0)

        # K: load natural chunk-major, cast to bf16, xbar-transpose pair
        # blocks. Transposing kb[:, j*128:(j+1)*128] (= chunks 2j | 2j+1 side
        # by side) yields K^T of chunk 2j on partitions 0-63 and of chunk
        # 2j+1 on partitions 64-127 -- the packed row-tiling layout.
        kn = loadp.tile([128, NKC * D], FP32, tag="kn")
        nc.sync.dma_start(
            kn[:].rearrange("p (c d) -> p c d", d=D),
            k.rearrange("(c p) d -> p c d", p=128),
        )
        kb = loadp.tile([128, NKC * D], H16, tag="kb")
        nc.vector.tensor_copy(kb[:], kn[:])
        kt2 = const.tile([128, NKC * D], H16)
        for j in range(NKC // 2):
            nc.sync.dma_start(
                kt2[:, j * 128 : (j + 1) * 128],
                kb[:, j * 128 : (j + 1) * 128],
                transpose=True,
            )

        # Q: same transpose trick into alternating halves, then rearrange
        # into qt2r with Q^T replicated on both partition halves.
        qn = loadp.tile([128, NQC * D], FP32, tag="qn")
        nc.sync.dma_start(
            qn[:].rearrange("p (c d) -> p c d", d=D),
            q.rearrange("(c p) d -> p c d", p=128),
        )
        qb = loadp.tile([128, NQC * D], H16, tag="qb")
        nc.vector.tensor_copy(qb[:], qn[:])
        qt_alt = loadp.tile([128, NQC * D], H16, tag="qt_alt")
        for j in range(NQC // 2):
            nc.sync.dma_start(
                qt_alt[:, j * 128 : (j + 1) * 128],
                qb[:, j * 128 : (j + 1) * 128],
                transpose=True,
            )
        qt2r = const.tile([128, LQ], H16)
        nblk = NQC // 2
        for parity in range(2):
            src = qt_alt[parity * 64 : (parity + 1) * 64, :].rearrange(
                "p (j c) -> p j c", c=128
            )
            for half in range(2):
                dst = qt2r[half * 64 : (half + 1) * 64, :].rearrange(
                    "p (j c) -> p j c", c=256
                )[:, :, parity * 128 : (parity + 1) * 128]
                nc.sync.dma_start(dst, src)

        if setup_only:
            return

        for ti in range(NQT):
            acc = ps_acc.tile([DP, QTW], FP32)
            ngrp = (NKC + GRP - 1) // GRP
            s_tiles = [None] * ngrp
            filled = [0] * ngrp
            qsl = slice(ti * QTW, (ti + 1) * QTW)

            def consume_group(g):
                gsz = min(GRP, NKC - g * GRP)
                p = pp.tile([128, GRP * QTW], FP32, tag="p")
                nc.scalar.activation(
                    p[:, : gsz * QTW], s_tiles[g][:, : gsz * QTW], Exp, scale=SCALE
                )
                for s in range(gsz):
                    c = g * GRP + s
                    nc.tensor.matmul(
                        acc[:],
                        vp[:, c * DP : (c + 1) * DP],
                        p[:, s * QTW : (s + 1) * QTW],
                        start=(c == 0),
                        stop=(c == NKC - 1),
                    )

            for c in range(NKC):
                g, slot = divmod(c, GRP)
                if s_tiles[g] is None:
                    s_tiles[g] = ps_scores.tile(
                        [128, GRP * QTW], FP32, tag="scores", name="sgrp"
                    )
                half, j = c % 2, c // 2
                nc.tensor.matmul(
                    s_tiles[g][:, slot * QTW : (slot + 1) * QTW],
                    kt2[half * 64 : (half + 1) * 64, j * 128 : (j + 1) * 128],
                    qt2r[half * 64 : (half + 1) * 64, qsl],
                    start=True,
                    stop=True,
                )
                filled[g] += 1
                if filled[g] == min(GRP, NKC - g * GRP):
                    consume_group(g)

            # Epilogue: transpose out^T back to [q, DP], divide by denom.
            accs = outp.tile([DP, QTW], FP32, tag="accs")
            nc.vector.tensor_copy(accs[:], acc[:])
            for j in range(QTW // 128):
                tr = ps_tr.tile([128, DP], FP32)
                nc.tensor.transpose(tr[:], accs[:, j * 128 : (j + 1) * 128],
                                    ident[:DP, :DP])
                rc = outp.tile([128, 1], FP32, tag="rc")
                nc.vector.reciprocal(rc[:], tr[:, D:DP])
                ot = outp.tile([128, D], FP32, tag="ot")
                nc.vector.tensor_scalar_mul(ot[:], tr[:, :D], rc[:])
                r0 = ti * QTW + j * 128
                nc.sync.dma_start(o[r0 : r0 + 128, :], ot[:])


_BUILT = {}

def emit_attention_v3(tc, q, k, v, o, LQ, LK, D, QTW, GRP=3, setup_only=False,
                      loop=None, pipelined=False, stage="full"):
    """v3: fp32 everywhere. Packed K^T layout (even chunks on partitions
    0-63, odd on 64-127) built with PE transposes (tile_position=(0,64)
    for the odd chunks); Q^T replicated to both halves with one
    SBUF->SBUF DMA. Row-tiled concurrent scores matmuls + GRP-chunk exp
    groups. `loop` (optional contextmanager factory) wraps the main loop
    for repeat-timing builds; setup stays outside.
    """
    import contextlib

    nc = tc.nc
    assert D == 64 and QTW % 128 == 0
    NKC = LK // 128
    NQC = LQ // 128
    NQT = LQ // QTW
    DP = D + 1
    SCALE = 1.0 / math.sqrt(D)
    Exp = mybir.ActivationFunctionType.Exp
    loop = loop or contextlib.nullcontext

    from contextlib import ExitStack

    with ExitStack() as ctx:
        const = ctx.enter_context(tc.tile_pool(name="const", bufs=1))
        loadp = ctx.enter_context(tc.tile_pool(name="load", bufs=1))
        pp = ctx.enter_context(tc.tile_pool(name="p", bufs=4))
        outp = ctx.enter_context(tc.tile_pool(name="outs", bufs=4))
        ps_scores = ctx.enter_context(
            tc.tile_pool(name="ps_scores", bufs=2, space="PSUM")
        )
        ps_acc = ctx.enter_context(tc.tile_pool(name="ps_acc", bufs=1, space="PSUM"))
        ps_tr = ctx.enter_context(tc.tile_pool(name="ps_tr", bufs=1, space="PSUM"))

        ident = const.tile([128, 128], FP32)
        make_identity(nc, ident[:])

        vp = const.tile([128, NKC * DP], FP32)
        vp3 = vp[:].rearrange("p (c w) -> p c w", w=DP)
        nc.sync.dma_start(vp3[:, :, 0:D], v.rearrange("(c p) d -> p c d", p=128))
        nc.gpsimd.memset(vp3[:, :, D:DP], 1.0)

        kn = loadp.tile([128, NKC * D], FP32, tag="kn")
        nc.sync.dma_start(
            kn[:].rearrange("p (c d) -> p c d", d=D),
            k.rearrange("(c p) d -> p c d", p=128),
        )
        qn = loadp.tile([128, NQC * D], FP32, tag="qn")
        nc.sync.dma_start(
            qn[:].rearrange("p (c d) -> p c d", d=D),
            q.rearrange("(c p) d -> p c d", p=128),
        )

        # Packed K^T: one [128,128] transpose per chunk PAIR. Transposing
        # kn[:, j*128:(j+1)*128] (chunks 2j | 2j+1 side by side) lands
        # chunk 2j's K^T on partitions 0-63 and chunk 2j+1's on 64-127.
        kt2 = const.tile([128, (NKC // 2) * 128], FP32)
        for j in range(NKC // 2):
            tr = ps_scores.tile([128, 128], FP32, tag="scores", name="trs")
            nc.tensor.transpose(tr[:], kn[:, j * 128 : (j + 1) * 128], ident[:])
            nc.vector.tensor_copy(kt2[:, j * 128 : (j + 1) * 128], tr[:])

        # Q^T on partitions 0-63, then replicate to 64-127 with one DMA.
        qt2r = const.tile([128, LQ], FP32)
        for c in range(NQC):
            tr = ps_scores.tile([128, 128], FP32, tag="scores", name="trs")
            nc.tensor.transpose(
                tr[0:64, :], qn[:, c * D : (c + 1) * D], ident[:]
            )
            nc.vector.tensor_copy(
                qt2r[0:64, c * 128 : (c + 1) * 128], tr[0:64, :]
            )
        nc.sync.dma_start(qt2r[64:128, :], qt2r[0:64, :])

        if setup_only:
            return

        with loop():
            for ti in range(NQT):
                acc = (
                    ps_acc.tile([DP, QTW], FP32, name="acc")
                    if stage in ("full", "noepi", "fakep")
                    else None
                )
                ngrp = (NKC + GRP - 1) // GRP
                s_tiles = [None] * ngrp
                qsl = slice(ti * QTW, (ti + 1) * QTW)

                def consume_group(g):
                    if stage == "mm1":
                        return
                    gsz = min(GRP, NKC - g * GRP)
                    p = pp.tile([128, GRP * QTW], FP32, tag="p", name="pg")
                    nc.scalar.activation(
                        p[:, : gsz * QTW], s_tiles[g][:, : gsz * QTW], Exp,
                        scale=SCALE,
                    )
                    if stage == "mm1exp":
                        return
                    for s in range(gsz):
                        c = g * GRP + s
                        rhs = (
                            qt2r[:, 0:QTW]
                            if stage == "fakep"
                            else p[:, s * QTW : (s + 1) * QTW]
                        )
                        nc.tensor.matmul(
                            acc[:],
                            vp[:, c * DP : (c + 1) * DP],
                            rhs,
                            start=(c == 0),
                            stop=(c == NKC - 1),
                        )

                pending = []
                for c in range(NKC):
                    g, slot = divmod(c, GRP)
                    if s_tiles[g] is None:
                        s_tiles[g] = ps_scores.tile(
                            [128, GRP * QTW], FP32, tag="scores", name="sgrp"
                        )
                    half, j = c % 2, c // 2
                    nc.tensor.matmul(
                        s_tiles[g][:, slot * QTW : (slot + 1) * QTW],
                        kt2[half * 64 : (half + 1) * 64, j * 128 : (j + 1) * 128],
                        qt2r[half * 64 : (half + 1) * 64, qsl],
                        start=True,
                        stop=True,
                    )
                    filled = c - g * GRP + 1
                    if filled == min(GRP, NKC - g * GRP):
                        if pipelined:
                            # delay exp+accumulate of group g until the next
                            # group's scores matmuls are in PE's stream, so PE
                            # never idles waiting on ScalarE's exp
                            pending.append(g)
                            if len(pending) > 1:
                                consume_group(pending.pop(0))
                        else:
                            consume_group(g)
                for g in pending:
                    consume_group(g)

                if stage != "full":
                    # keep every engine + DMA alive inside the loop body --
                    # an engine with zero loop instructions deadlocks the
                    # For_i back-edge barrier (observed: device wedge).
                    ka = outp.tile([128, 4], FP32, tag="ka", name="ka")
                    nc.gpsimd.memset(ka[:], 0.0)
                    kb_ = outp.tile([128, 4], FP32, tag="kb", name="kb")
                    nc.vector.tensor_copy(kb_[:], ka[:])
                    kc_ = outp.tile([128, 4], FP32, tag="kc", name="kc")
                    nc.scalar.mul(kc_[:], kb_[:], 1.0)
                    nc.sync.dma_start(o[ti * 128 : ti * 128 + 128, 0:4], kc_[:])
                    continue
                accs = outp.tile([DP, QTW], FP32, tag="accs")
                nc.vector.tensor_copy(accs[:], acc[:])
                for j in range(QTW // 128):
                    tr = ps_tr.tile([128, DP], FP32)
                    nc.tensor.transpose(tr[:], accs[:, j * 128 : (j + 1) * 128],
                                        ident[:DP, :DP])
                    rc = outp.tile([128, 1], FP32, tag="rc")
                    nc.vector.reciprocal(rc[:], tr[:, D:DP])
                    ot = outp.tile([128, D], FP32, tag="ot")
                    nc.vector.tensor_scalar_mul(ot[:], tr[:, :D], rc[:])
                    r0 = ti * QTW + j * 128
                    nc.sync.dma_start(o[r0 : r0 + 128, :], ot[:])


import functools

EMITTERS = {
    "v1": emit_attention,
    "v1t": functools.partial(emit_attention, GRP=3),
    "v1set": functools.partial(emit_attention, setup_only=True),
    "v2": emit_attention_v2,
    "v2g2": functools.partial(emit_attention_v2, GRP=2),
    "v2set": functools.partial(emit_attention_v2, setup_only=True),
    "v3": emit_attention_v3,
    "v3p": functools.partial(emit_attention_v3, pipelined=True),
    "v3g2": functools.partial(emit_attention_v3, GRP=2),
    "v3set": functools.partial(emit_attention_v3, setup_only=True),
    "v3mm1": functools.partial(emit_attention_v3, stage="mm1"),
    "v3mm1e": functools.partial(emit_attention_v3, stage="mm1exp"),
    "v3mm1ep": functools.partial(emit_attention_v3, stage="mm1exp",
                                 pipelined=True),
    "v3noepi": functools.partial(emit_attention_v3, stage="noepi",
                                 pipelined=True),
    "v3fakep": functools.partial(emit_attention_v3, stage="fakep",
                                 pipelined=True),
}
LOOP_SPLIT = {"v3", "v3p", "v3g2", "v3mm1", "v3mm1e", "v3mm1ep",
              "v3noepi", "v3fakep"}
DEFAULT_VARIANT = "v3p"


def _build(repeat=1, variant=None):
    """Build the per-core module. repeat>1 wraps the body in a hardware
    For_i loop (used only for on-device timing; grading uses repeat=1)."""
    variant = variant or DEFAULT_VARIANT
    key = (repeat, variant)
    if key not in _BUILT:
        emitter = EMITTERS[variant]
        nc = bacc.Bacc("TRN2", target_bir_lowering=False, debug=False)
        q = nc.dram_tensor("q", [LQ, D], FP32, kind="ExternalInput")
        k = nc.dram_tensor("k", [LK, D], FP32, kind="ExternalInput")
        v = nc.dram_tensor("v", [LK, D], FP32, kind="ExternalInput")
        o = nc.dram_tensor("o", [LQ, D], FP32, kind="ExternalOutput")
        engines = (
            mybir.EngineType.PE,
            mybir.EngineType.Activation,
            mybir.EngineType.DVE,
            mybir.EngineType.SP,
            mybir.EngineType.Pool,
        )
        with tile.TileContext(nc) as tc:
            if repeat == 1:
                emitter(tc, q[:], k[:], v[:], o[:], LQ, LK, D, QTW=512)
            elif variant in LOOP_SPLIT:
                emitter(
                    tc, q[:], k[:], v[:], o[:], LQ, LK, D, QTW=512,
                    loop=lambda: tc.For_i(0, repeat, 1, hint_engines=engines),
                )
            else:
                with tc.For_i(0, repeat, 1, hint_engines=engines):
                    emitter(tc, q[:], k[:], v[:], o[:], LQ, LK, D, QTW=512)
        nc.compile()
        _BUILT[key] = nc
    return _BUILT[key]


def _shard_inputs(query, keys, values):
    in_maps = []
    for c in range(N_CORES):
        b, h = c // 2, c % 2
        in_maps.append({
            "q": np.ascontiguousarray(query[b, h * LQ : (h + 1) * LQ, :],
                                      dtype=np.float32),
            "k": np.ascontiguousarray(keys[b], dtype=np.float32),
            "v": np.ascontiguousarray(values[b], dtype=np.float32),
        })
    return in_maps


def run_sharded(query, keys, values, trace=False, repeat=1, variant=None):
    """Run on 8 cores; returns (full_output, BassKernelResults)."""
    from concourse.bass_utils import run_bass_kernel_spmd

    nc = _build(repeat, variant)
    in_maps = _shard_inputs(query, keys, values)
    res = run_bass_kernel_spmd(nc, in_maps, list(range(N_CORES)), trace=trace)
    out = np.empty((B, LQ_FULL, D), np.float32)
    for c in range(N_CORES):
        b, h = c // 2, c % 2
        out[b, h * LQ : (h + 1) * LQ, :] = res.results[c]["o"]
    return out, res


def kernel(query, keys, values):
    out, _ = run_sharded(np.asarray(query), np.asarray(keys), np.asarray(values))
    return out


# revision 44
# speedup vs baseline: 2.0442x; 1.2342x over previous
"""Trainium2 Bass kernel for batched dot-product attention.

Problem: query/keys/values [4, 4096, 64] fp32 -> softmax(Q K^T / 8) V.

Sharding: 8 cores, data parallel. Core c handles batch c//2, query rows
(c%2)*2048 : (c%2+1)*2048, and needs full K/V of its batch. Each core runs
an identical program (SPMD) on its own shard.

Per-core algorithm (flash-attention-like, no max subtraction -- scores are
~N(0,1) after the 1/sqrt(64) scale so exp() cannot overflow):
  - K^T [64, 4096] and Q^T [64, 2048] built via PE transposes (contraction
    dim d must sit on partitions for the QK^T matmul).
  - V' = [V | ones] chunks [128, 65]: the ones column makes the second
    matmul emit the softmax denominator as row 64 of the output.
  - For each 512-wide q tile: 32 matmuls produce scoresT [k=128, q=512]
    chunks in PSUM; ScalarE applies exp (scale folded in); second matmul
    accumulates V'^T @ P into out^T [65, 512]; epilogue transposes back to
    [q, 65], divides by the denominator column and DMAs out.
"""

import math

import numpy as np

import concourse.bass as bass
import concourse.tile as tile
from concourse import bacc, mybir
from concourse.masks import make_identity

FP32 = mybir.dt.float32

# Production shard dims
B, LQ_FULL, LK, D = 4, 4096, 4096, 64
N_CORES = 8
LQ = LQ_FULL * B // N_CORES  # 2048 query rows per core


def emit_attention(tc, q, k, v, o, LQ, LK, D, QTW, GRP=2, setup_only=False):
    """Emit the per-core attention program into TileContext tc.

    q: [LQ, D], k: [LK, D], v: [LK, D] DRAM inputs; o: [LQ, D] DRAM output.
    QTW: q-tile width (free dim of the scores matmul), <= 512 for fp32.
    GRP: k-chunks per exp group (2 -> 4+2+2 PSUM banks, 3 -> 6+1+1).
    """
    nc = tc.nc
    NKC = LK // 128   # key chunks
    NQC = LQ // 128   # query chunks (for transposes)
    NQT = LQ // QTW   # q tiles
    DP = D + 1        # V width with the ones column
    SCALE = 1.0 / math.sqrt(D)
    Exp = mybir.ActivationFunctionType.Exp
    accbufs = 2 if GRP == 2 else 1

    from contextlib import ExitStack

    with ExitStack() as ctx:
        const = ctx.enter_context(tc.tile_pool(name="const", bufs=1))
        kq = ctx.enter_context(tc.tile_pool(name="kq", bufs=1))
        loadp = ctx.enter_context(tc.tile_pool(name="load", bufs=1))
        pp = ctx.enter_context(tc.tile_pool(name="p", bufs=4))
        outp = ctx.enter_context(tc.tile_pool(name="outs", bufs=4))
        ps_scores = ctx.enter_context(
            tc.tile_pool(name="ps_scores", bufs=2, space="PSUM")
        )
        ps_acc = ctx.enter_context(
            tc.tile_pool(name="ps_acc", bufs=accbufs, space="PSUM")
        )
        ps_tr = ctx.enter_context(
            tc.tile_pool(name="ps_tr", bufs=accbufs, space="PSUM")
        )

        ident = const.tile([128, 128], FP32)
        make_identity(nc, ident[:])

        # V' = [V | 1] chunks, [128, DP] each, packed side by side.
        # One DMA for all of V, one strided memset for the ones columns.
        vp = const.tile([128, NKC * DP], FP32)
        vp3 = vp[:].rearrange("p (c w) -> p c w", w=DP)
        nc.sync.dma_start(vp3[:, :, 0:D], v.rearrange("(c p) d -> p c d", p=128))
        nc.gpsimd.memset(vp3[:, :, D:DP], 1.0)

        # Natural-layout K/Q staged chunk-major with one DMA each.
        kn = loadp.tile([128, NKC * D], FP32, tag="kn")
        nc.sync.dma_start(
            kn[:].rearrange("p (c d) -> p c d", d=D),
            k.rearrange("(c p) d -> p c d", p=128),
        )
        qn = loadp.tile([128, NQC * D], FP32, tag="qn")
        nc.sync.dma_start(
            qn[:].rearrange("p (c d) -> p c d", d=D),
            q.rearrange("(c p) d -> p c d", p=128),
        )

        # K^T / Q^T via PE transposes of the natural-layout chunks.
        kt = kq.tile([D, LK], FP32)
        for c in range(NKC):
            tr = ps_scores.tile([D, 128], FP32, tag="scores")
            nc.tensor.transpose(tr[:], kn[:, c * D : (c + 1) * D], ident[:])
            nc.vector.tensor_copy(kt[:, c * 128 : (c + 1) * 128], tr[:])

        qt = kq.tile([D, LQ], FP32)
        for c in range(NQC):
            tr = ps_scores.tile([D, 128], FP32, tag="scores")
            nc.tensor.transpose(tr[:], qn[:, c * D : (c + 1) * D], ident[:])
            nc.vector.tensor_copy(qt[:, c * 128 : (c + 1) * 128], tr[:])

        if setup_only:
            return

        for ti in range(NQT):
            acc = ps_acc.tile([DP, QTW], FP32)
            ngrp = (NKC + GRP - 1) // GRP
            qs = qt[:, ti * QTW : (ti + 1) * QTW]
            for g in range(ngrp):
                gsz = min(GRP, NKC - g * GRP)
                s = ps_scores.tile([128, GRP * QTW], FP32, tag="scores")
                for i in range(gsz):
                    c = g * GRP + i
                    nc.tensor.matmul(
                        s[:, i * QTW : (i + 1) * QTW],
                        kt[:, c * 128 : (c + 1) * 128], qs,
                        start=True, stop=True,
                    )
                p = pp.tile([128, GRP * QTW], FP32, tag="p")
                nc.scalar.activation(
                    p[:, : gsz * QTW], s[:, : gsz * QTW], Exp, scale=SCALE
                )
                for i in range(gsz):
                    c = g * GRP + i
                    nc.tensor.matmul(
                        acc[:], vp[:, c * DP : (c + 1) * DP],
                        p[:, i * QTW : (i + 1) * QTW],
                        start=(c == 0), stop=(c == NKC - 1),
                    )

            # Epilogue: transpose out^T back to [q, DP], divide by denominator.
            accs = outp.tile([DP, QTW], FP32, tag="accs")
            nc.vector.tensor_copy(accs[:], acc[:])
            for j in range(QTW // 128):
                tr = ps_tr.tile([128, DP], FP32)
                nc.tensor.transpose(tr[:], accs[:, j * 128 : (j + 1) * 128],
                                    ident[:DP, :DP])
                rc = outp.tile([128, 1], FP32, tag="rc")
                nc.vector.reciprocal(rc[:], tr[:, D:DP])
                ot = outp.tile([128, D], FP32, tag="ot")
                nc.vector.tensor_scalar_mul(ot[:], tr[:, :D], rc[:])
                r0 = ti * QTW + j * 128
                nc.sync.dma_start(o[r0 : r0 + 128, :], ot[:])


H16 = mybir.dt.float16


def emit_attention_v2(tc, q, k, v, o, LQ, LK, D, QTW, GRP=3, setup_only=False):
    """v2: bf16 K^T/Q^T via DMA xbar transposes in a packed layout
    (even k-chunks on partitions 0-63, odd on 64-127) + row-tiled
    concurrent scores matmuls + 3-chunk exp groups.

    Requires D == 64 (pair-packing trick) and QTW == 512.
    """
    nc = tc.nc
    assert D == 64 and QTW % 128 == 0
    NKC = LK // 128   # k chunks
    NQC = LQ // 128   # q chunks
    NQT = LQ // QTW
    DP = D + 1
    GRP = 3           # chunks per exp group (3 PSUM banks)
    SCALE = 1.0 / math.sqrt(D)
    Exp = mybir.ActivationFunctionType.Exp

    from contextlib import ExitStack

    with ExitStack() as ctx:
        const = ctx.enter_context(tc.tile_pool(name="const", bufs=1))
        loadp = ctx.enter_context(tc.tile_pool(name="load", bufs=1))
        pp = ctx.enter_context(tc.tile_pool(name="p", bufs=4))
        outp = ctx.enter_context(tc.tile_pool(name="outs", bufs=4))
        ps_scores = ctx.enter_context(
            tc.tile_pool(name="ps_scores", bufs=2, space="PSUM")
        )
        ps_acc = ctx.enter_context(tc.tile_pool(name="ps_acc", bufs=1, space="PSUM"))
        ps_tr = ctx.enter_context(tc.tile_pool(name="ps_tr", bufs=1, space="PSUM"))

        ident = const.tile([128, 128], FP32)
        make_identity(nc, ident[:])

        # V' = [V | 1] chunks (fp32; value path stays fp32).
        vp = const.tile([128, NKC * DP], FP32)
        vp3 = vp[:].rearrange("p (c w) -> p c w", w=DP)
        nc.sync.dma_start(vp3[:, :, 0:D], v.rearrange("(c p) d -> p c d", p=128))
        nc.gpsimd.memset(vp3[:, :, D:DP], 1.0)

        # K: load natural chunk-major, cast to bf16, xbar-transpose pair
        # blocks. Transposing kb[:, j*128:(j+1)*128] (= chunks 2j | 2j+1 side
        # by side) yields K^T of chunk 2j on partitions 0-63 and of chunk
        # 2j+1 on partitions 64-127 -- the packed row-tiling layout.
        kn = loadp.tile([128, NKC * D], FP32, tag="kn")
        nc.sync.dma_start(
            kn[:].rearrange("p (c d) -> p c d", d=D),
            k.rearrange("(c p) d -> p c d", p=128),
        )
        kb = loadp.tile([128, NKC * D], H16, tag="kb")
        nc.vector.tensor_copy(kb[:], kn[:])
        kt2 = const.tile([128, NKC * D], H16)
        for j in range(NKC // 2):
            nc.sync.dma_start(
                kt2[:, j * 128 : (j + 1) * 128],
                kb[:, j * 128 : (j + 1) * 128],
                transpose=True,
            )

        # Q: same transpose trick into alternating halves, then rearrange
        # into qt2r with Q^T replicated on both partition halves.
        qn = loadp.tile([128, NQC * D], FP32, tag="qn")
        nc.sync.dma_start(
            qn[:].rearrange("p (c d) -> p c d", d=D),
            q.rearrange("(c p) d -> p c d", p=128),
        )
        qb = loadp.tile([128, NQC * D], H16, tag="qb")
        nc.vector.tensor_copy(qb[:], qn[:])
        qt_alt = loadp.tile([128, NQC * D], H16, tag="qt_alt")
        for j in range(NQC // 2):
            nc.sync.dma_start(
                qt_alt[:, j * 128 : (j + 1) * 128],
                qb[:, j * 128 : (j + 1) * 128],
                transpose=True,
            )
        qt2r = const.tile([128, LQ], H16)
        nblk = NQC // 2
        for parity in range(2):
            src = qt_alt[parity * 64 : (parity + 1) * 64, :].rearrange(
                "p (j c) -> p j c", c=128
            )
            for half in range(2):
                dst = qt2r[half * 64 : (half + 1) * 64, :].rearrange(
                    "p (j c) -> p j c", c=256
                )[:, :, parity * 128 : (parity + 1) * 128]
                nc.sync.dma_start(dst, src)

        if setup_only:
            return

        for ti in range(NQT):
            acc = ps_acc.tile([DP, QTW], FP32)
            ngrp = (NKC + GRP - 1) // GRP
            s_tiles = [None] * ngrp
            filled = [0] * ngrp
            qsl = slice(ti * QTW, (ti + 1) * QTW)

            def consume_group(g):
                gsz = min(GRP, NKC - g * GRP)
                p = pp.tile([128, GRP * QTW], FP32, tag="p")
                nc.scalar.activation(
                    p[:, : gsz * QTW], s_tiles[g][:, : gsz * QTW], Exp, scale=SCALE
                )
                for s in range(gsz):
                    c = g * GRP + s
                    nc.tensor.matmul(
                        acc[:],
                        vp[:, c * DP : (c + 1) * DP],
                        p[:, s * QTW : (s + 1) * QTW],
                        start=(c == 0),
                        stop=(c == NKC - 1),
                    )

            for c in range(NKC):
                g, slot = divmod(c, GRP)
                if s_tiles[g] is None:
                    s_tiles[g] = ps_scores.tile(
                        [128, GRP * QTW], FP32, tag="scores", name="sgrp"
                    )
                half, j = c % 2, c // 2
                nc.tensor.matmul(
                    s_tiles[g][:, slot * QTW : (slot + 1) * QTW],
                    kt2[half * 64 : (half + 1) * 64, j * 128 : (j + 1) * 128],
                    qt2r[half * 64 : (half + 1) * 64, qsl],
                    start=True,
                    stop=True,
                )
                filled[g] += 1
                if filled[g] == min(GRP, NKC - g * GRP):
                    consume_group(g)

            # Epilogue: transpose out^T back to [q, DP], divide by denom.
            accs = outp.tile([DP, QTW], FP32, tag="accs")
            nc.vector.tensor_copy(accs[:], acc[:])
            for j in range(QTW // 128):
                tr = ps_tr.tile([128, DP], FP32)
                nc.tensor.transpose(tr[:], accs[:, j * 128 : (j + 1) * 128],
                                    ident[:DP, :DP])
                rc = outp.tile([128, 1], FP32, tag="rc")
                nc.vector.reciprocal(rc[:], tr[:, D:DP])
                ot = outp.tile([128, D], FP32, tag="ot")
                nc.vector.tensor_scalar_mul(ot[:], tr[:, :D], rc[:])
                r0 = ti * QTW + j * 128
                nc.sync.dma_start(o[r0 : r0 + 128, :], ot[:])


_BUILT = {}

def emit_attention_v3(tc, q, k, v, o, LQ, LK, D, QTW, GRP=3, setup_only=False,
                      loop=None, pipelined=False, stage="full", pbufs=4,
                      sbufs=2, depth=1):
    """v3: fp32 everywhere. Packed K^T layout (even chunks on partitions
    0-63, odd on 64-127) built with PE transposes (tile_position=(0,64)
    for the odd chunks); Q^T replicated to both halves with one
    SBUF->SBUF DMA. Row-tiled concurrent scores matmuls + GRP-chunk exp
    groups. `loop` (optional contextmanager factory) wraps the main loop
    for repeat-timing builds; setup stays outside.
    """
    import contextlib

    nc = tc.nc
    assert D == 64 and QTW % 128 == 0
    NKC = LK // 128
    NQC = LQ // 128
    NQT = LQ // QTW
    DP = D + 1
    SCALE = 1.0 / math.sqrt(D)
    Exp = mybir.ActivationFunctionType.Exp
    loop = loop or contextlib.nullcontext

    from contextlib import ExitStack

    with ExitStack() as ctx:
        const = ctx.enter_context(tc.tile_pool(name="const", bufs=1))
        loadp = ctx.enter_context(tc.tile_pool(name="load", bufs=1))
        pp = ctx.enter_context(tc.tile_pool(name="p", bufs=pbufs))
        outp = ctx.enter_context(tc.tile_pool(name="outs", bufs=4))
        ps_scores = ctx.enter_context(
            tc.tile_pool(name="ps_scores", bufs=sbufs, space="PSUM")
        )
        ps_acc = ctx.enter_context(tc.tile_pool(name="ps_acc", bufs=1, space="PSUM"))
        ps_tr = ctx.enter_context(tc.tile_pool(name="ps_tr", bufs=1, space="PSUM"))

        ident = const.tile([128, 128], FP32)
        make_identity(nc, ident[:])

        vp = const.tile([128, NKC * DP], FP32)
        vp3 = vp[:].rearrange("p (c w) -> p c w", w=DP)
        nc.sync.dma_start(vp3[:, :, 0:D], v.rearrange("(c p) d -> p c d", p=128))
        nc.gpsimd.memset(vp3[:, :, D:DP], 1.0)

        kn = loadp.tile([128, NKC * D], FP32, tag="kn")
        nc.sync.dma_start(
            kn[:].rearrange("p (c d) -> p c d", d=D),
            k.rearrange("(c p) d -> p c d", p=128),
        )
        qn = loadp.tile([128, NQC * D], FP32, tag="qn")
        nc.sync.dma_start(
            qn[:].rearrange("p (c d) -> p c d", d=D),
            q.rearrange("(c p) d -> p c d", p=128),
        )

        # Packed K^T: one [128,128] transpose per chunk PAIR. Transposing
        # kn[:, j*128:(j+1)*128] (chunks 2j | 2j+1 side by side) lands
        # chunk 2j's K^T on partitions 0-63 and chunk 2j+1's on 64-127.
        kt2 = const.tile([128, (NKC // 2) * 128], FP32)
        for j in range(NKC // 2):
            tr = ps_scores.tile([128, 128], FP32, tag="scores", name="trs")
            nc.tensor.transpose(tr[:], kn[:, j * 128 : (j + 1) * 128], ident[:])
            nc.vector.tensor_copy(kt2[:, j * 128 : (j + 1) * 128], tr[:])

        # Q^T on partitions 0-63, then replicate to 64-127 with one DMA.
        qt2r = const.tile([128, LQ], FP32)
        for c in range(NQC):
            tr = ps_scores.tile([128, 128], FP32, tag="scores", name="trs")
            nc.tensor.transpose(
                tr[0:64, :], qn[:, c * D : (c + 1) * D], ident[:]
            )
            nc.vector.tensor_copy(
                qt2r[0:64, c * 128 : (c + 1) * 128], tr[0:64, :]
            )
        nc.sync.dma_start(qt2r[64:128, :], qt2r[0:64, :])

        if setup_only:
            return

        with loop():
            for ti in range(NQT):
                acc = (
                    ps_acc.tile([DP, QTW], FP32, name="acc")
                    if stage in ("full", "noepi", "fakep")
                    else None
                )
                ngrp = (NKC + GRP - 1) // GRP
                s_tiles = [None] * ngrp
                qsl = slice(ti * QTW, (ti + 1) * QTW)

                def consume_group(g):
                    if stage == "mm1":
                        return
                    gsz = min(GRP, NKC - g * GRP)
                    p = pp.tile([128, GRP * QTW], FP32, tag="p", name="pg")
                    nc.scalar.activation(
                        p[:, : gsz * QTW], s_tiles[g][:, : gsz * QTW], Exp,
                        scale=SCALE,
                    )
                    if stage == "mm1exp":
                        return
                    for s in range(gsz):
                        c = g * GRP + s
                        rhs = (
                            qt2r[:, 0:QTW]
                            if stage == "fakep"
                            else p[:, s * QTW : (s + 1) * QTW]
                        )
                        nc.tensor.matmul(
                            acc[:],
                            vp[:, c * DP : (c + 1) * DP],
                            rhs,
                            start=(c == 0),
                            stop=(c == NKC - 1),
                        )

                pending = []
                for c in range(NKC):
                    g, slot = divmod(c, GRP)
                    if s_tiles[g] is None:
                        s_tiles[g] = ps_scores.tile(
                            [128, GRP * QTW], FP32, tag="scores", name="sgrp"
                        )
                    half, j = c % 2, c // 2
                    nc.tensor.matmul(
                        s_tiles[g][:, slot * QTW : (slot + 1) * QTW],
                        kt2[half * 64 : (half + 1) * 64, j * 128 : (j + 1) * 128],
                        qt2r[half * 64 : (half + 1) * 64, qsl],
                        start=True,
                        stop=True,
                    )
                    filled = c - g * GRP + 1
                    if filled == min(GRP, NKC - g * GRP):
                        if pipelined:
                            # delay exp+accumulate of group g until later
                            # groups' scores matmuls are in PE's stream, so PE
                            # never idles waiting on ScalarE's exp
                            pending.append(g)
                            if len(pending) > depth:
                                consume_group(pending.pop(0))
                        else:
                            consume_group(g)
                for g in pending:
                    consume_group(g)

                if stage != "full":
                    # keep every engine + DMA alive inside the loop body --
                    # an engine with zero loop instructions deadlocks the
                    # For_i back-edge barrier (observed: device wedge).
                    ka = outp.tile([128, 4], FP32, tag="ka", name="ka")
                    nc.gpsimd.memset(ka[:], 0.0)
                    kb_ = outp.tile([128, 4], FP32, tag="kb", name="kb")
                    nc.vector.tensor_copy(kb_[:], ka[:])
                    kc_ = outp.tile([128, 4], FP32, tag="kc", name="kc")
                    nc.scalar.mul(kc_[:], kb_[:], 1.0)
                    nc.sync.dma_start(o[ti * 128 : ti * 128 + 128, 0:4], kc_[:])
                    continue
                accs = outp.tile([DP, QTW], FP32, tag="accs")
                nc.vector.tensor_copy(accs[:], acc[:])
                for j in range(QTW // 128):
                    tr = ps_tr.tile([128, DP], FP32)
                    nc.tensor.transpose(tr[:], accs[:, j * 128 : (j + 1) * 128],
                                        ident[:DP, :DP])
                    rc = outp.tile([128, 1], FP32, tag="rc")
                    nc.vector.reciprocal(rc[:], tr[:, D:DP])
                    ot = outp.tile([128, D], FP32, tag="ot")
                    nc.vector.tensor_scalar_mul(ot[:], tr[:, :D], rc[:])
                    r0 = ti * QTW + j * 128
                    nc.sync.dma_start(o[r0 : r0 + 128, :], ot[:])


import functools

EMITTERS = {
    "v1": emit_attention,
    "v1t": functools.partial(emit_attention, GRP=3),
    "v1set": functools.partial(emit_attention, setup_only=True),
    "v2": emit_attention_v2,
    "v2g2": functools.partial(emit_attention_v2, GRP=2),
    "v2set": functools.partial(emit_attention_v2, setup_only=True),
    "v3": emit_attention_v3,
    "v3p": functools.partial(emit_attention_v3, pipelined=True),
    "v3g2": functools.partial(emit_attention_v3, GRP=2),
    "v3set": functools.partial(emit_attention_v3, setup_only=True),
    "v3mm1": functools.partial(emit_attention_v3, stage="mm1"),
    "v3mm1e": functools.partial(emit_attention_v3, stage="mm1exp"),
    "v3mm1ep": functools.partial(emit_attention_v3, stage="mm1exp",
                                 pipelined=True),
    "v3noepi": functools.partial(emit_attention_v3, stage="noepi",
                                 pipelined=True),
    "v3fakep": functools.partial(emit_attention_v3, stage="fakep",
                                 pipelined=True),
    "v3pb": functools.partial(emit_attention_v3, pipelined=True, pbufs=8),
    "v4": functools.partial(emit_attention_v3, GRP=2, sbufs=3, depth=2,
                            pbufs=6, pipelined=True),
    "v4d1": functools.partial(emit_attention_v3, GRP=2, sbufs=3, depth=1,
                              pbufs=6, pipelined=True),
}
LOOP_SPLIT = {"v3", "v3p", "v3g2", "v3mm1", "v3mm1e", "v3mm1ep",
              "v3noepi", "v3fakep", "v3pb", "v4", "v4d1"}
DEFAULT_VARIANT = "v4d1"


def _build(repeat=1, variant=None):
    """Build the per-core module. repeat>1 wraps the body in a hardware
    For_i loop (used only for on-device timing; grading uses repeat=1)."""
    variant = variant or DEFAULT_VARIANT
    key = (repeat, variant)
    if key not in _BUILT:
        emitter = EMITTERS[variant]
        nc = bacc.Bacc("TRN2", target_bir_lowering=False, debug=False)
        q = nc.dram_tensor("q", [LQ, D], FP32, kind="ExternalInput")
        k = nc.dram_tensor("k", [LK, D], FP32, kind="ExternalInput")
        v = nc.dram_tensor("v", [LK, D], FP32, kind="ExternalInput")
        o = nc.dram_tensor("o", [LQ, D], FP32, kind="ExternalOutput")
        engines = (
            mybir.EngineType.PE,
            mybir.EngineType.Activation,
            mybir.EngineType.DVE,
            mybir.EngineType.SP,
            mybir.EngineType.Pool,
        )
        with tile.TileContext(nc) as tc:
            if repeat == 1:
                emitter(tc, q[:], k[:], v[:], o[:], LQ, LK, D, QTW=512)
            elif variant in LOOP_SPLIT:
                emitter(
                    tc, q[:], k[:], v[:], o[:], LQ, LK, D, QTW=512,
                    loop=lambda: tc.For_i(0, repeat, 1, hint_engines=engines),
                )
            else:
                with tc.For_i(0, repeat, 1, hint_engines=engines):
                    emitter(tc, q[:], k[:], v[:], o[:], LQ, LK, D, QTW=512)
        nc.compile()
        _BUILT[key] = nc
    return _BUILT[key]


def _shard_inputs(query, keys, values):
    in_maps = []
    for c in range(N_CORES):
        b, h = c // 2, c % 2
        in_maps.append({
            "q": np.ascontiguousarray(query[b, h * LQ : (h + 1) * LQ, :],
                                      dtype=np.float32),
            "k": np.ascontiguousarray(keys[b], dtype=np.float32),
            "v": np.ascontiguousarray(values[b], dtype=np.float32),
        })
    return in_maps


def run_sharded(query, keys, values, trace=False, repeat=1, variant=None):
    """Run on 8 cores; returns (full_output, BassKernelResults)."""
    from concourse.bass_utils import run_bass_kernel_spmd

    nc = _build(repeat, variant)
    in_maps = _shard_inputs(query, keys, values)
    res = run_bass_kernel_spmd(nc, in_maps, list(range(N_CORES)), trace=trace)
    out = np.empty((B, LQ_FULL, D), np.float32)
    for c in range(N_CORES):
        b, h = c // 2, c % 2
        out[b, h * LQ : (h + 1) * LQ, :] = res.results[c]["o"]
    return out, res


def kernel(query, keys, values):
    out, _ = run_sharded(np.asarray(query), np.asarray(keys), np.asarray(values))
    return out
